# revision 25
# baseline (speedup 1.0000x reference)
"""Trainium2 Bass kernel for nn_EnhancedChunkLayer (ragged_sequence).

Strategy: data-parallel over batch B=8 across 8 NeuronCores (one batch
element per core, weights replicated). Inside each core (all PE matmul
operands fp16; fp32 PSUM accumulation, softmax and LayerNorm math):

  - banded block-diagonal attention with a centered wt*128-wide column
    window; scores are computed TRANSPOSED (scoresT[wincol, row] =
    K_h-layout @ Q_h) so the exp'd probs feed attn@V directly as the
    stationary operand -- no PE transposes at all.
  - softmax: exp directly off the scores PSUM (ACT), then a MULTIPLICATIVE
    {0,1} fp16 mask on DVE (exp(s+m) == exp(s)*exp(m)); V carries an
    interleaved 129th ones-column per head so the denominator accumulates
    inside the attn@V matmul itself; per-(t) batched max(den,eps) +
    reciprocal on DVE via a strided AP over the den columns; the per-row
    1/den folds into the per-head ctx PSUM->SBUF copy (Pool engine,
    per-partition scalar).
  - QK-projection of head-group g+1 is emission-interleaved with the
    attention of group g, so the in-order PE always has projection matmuls
    to chew on while ACT/DVE run the softmax chain.
  - mean-pool FIRST (on ctx, via a host-built recip-scaled one-hot matmul),
    THEN the out-projection on [MAXC, D] instead of [S, D] (4x fewer rows).
    bv folds into ob' = bv @ Wo^T + ob on the host; ob', the size-embedding
    lookup (host gather) and pos_enc fold into one addvecT tensor.
  - chunk MLP streams W1/W2 in combined per-n chunks with h1(n) / h2(n-1)
    software-pipelined; exact-erf GELU on ACT; LayerNorm on chip.

Host does index bookkeeping only: cumsum of boundary indicators, mask /
scaled-one-hot construction, the size-embedding row gather, bias folds, and
weight layout packs (fp16).
"""

import math
from contextlib import ExitStack

import numpy as np

import concourse.bacc as bacc
import concourse.bass as bass
import concourse.mybir as mybir
from concourse import tile
from concourse.bass_utils import run_bass_kernel_spmd

F32 = mybir.dt.float32
F16 = mybir.dt.float16
F8 = mybir.dt.float8e4
DR = mybir.MatmulPerfMode.DoubleRow
AF = mybir.ActivationFunctionType
ALU = mybir.AluOpType
AX = mybir.AxisListType

B, S, D = 8, 1024, 1536
H, DH = 12, 128
MAXC, MAXSEQ = 256, 1024
THRESH = 0.85
P = 128
KD = D // P          # 12 contraction tiles over D
NT = S // P          # 8 row tiles over S
N2 = (2 * D) // P    # 24 tiles over hidden 2D
CT = MAXC // P       # 2 chunk tiles
DD3 = D // 512       # 3 free-dim 512 tiles over D
HG = 3               # heads per group
NG = H // HG         # 4 head groups
VW = 129             # per-head V block: 128 feats + 1 ones col (den)
HB = H * VW          # per-tile V block width
W12 = KD * P + D     # combined per-n W1 col block + W2 row block
MCH = 2              # n-tiles per streamed MLP chunk
DK2 = KD // 2        # 6 double-contraction tiles (K=256, fp8 DoubleRow)
INV_SD = 1.0 / math.sqrt(DH)
EPS_DEN = 2.0 ** -14  # fp16 min normal; keeps fully-masked rows finite


def _q8(a, target=160.0):
    """Quantize to e4m3 with a power-of-2 scale; returns (quant, scale)."""
    a = np.asarray(a, np.float32)
    m = float(np.max(np.abs(a))) or 1.0
    sc = 2.0 ** np.floor(np.log2(target / m))
    return np.ascontiguousarray((a * sc).astype(mybir.dt.np(F8))), sc


def _strided_ap(base, extra_off, dims):
    """AP over `base`'s tensor at base.offset+extra_off with free dims
    `dims` (list of [stride, count]); partition dim copied from base."""
    return bass.AP(tensor=base.tensor, offset=base.offset + extra_off,
                   ap=[list(base.ap[0])] + [list(d) for d in dims])


# ---------------------------------------------------------------- host prep

def _host_segments(boundaries_b):
    is_b = boundaries_b > THRESH
    seg = np.cumsum(is_b.astype(np.int64)) - 1
    valid = seg >= 0
    seg_c = np.where(valid & (seg < MAXC), seg, MAXC)
    lengths = np.bincount(seg_c, minlength=MAXC + 1)[:MAXC]
    return seg, valid, seg_c, lengths


def _window_tiles(seg_list):
    """Smallest odd tile-count window covering every chunk from any row tile."""
    wt = 3
    while True:
        if wt > NT:
            return NT
        pad = (wt - 1) // 2
        ok = True
        for seg in seg_list:
            for t in range(NT):
                ci = min(max(t - pad, 0), NT - wt)
                lo, hi = ci * P, ci * P + wt * P
                rows = np.arange(t * P, (t + 1) * P)
                segs = seg[rows]
                vmask = segs >= 0
                if not vmask.any():
                    continue
                cols = np.isin(seg, segs[vmask]) & (seg >= 0)
                idx = np.nonzero(cols)[0]
                if len(idx) and (idx[0] < lo or idx[-1] >= hi):
                    ok = False
                    break
            if not ok:
                break
        if ok:
            return wt
        wt += 2


def _host_per_batch(seg, valid, seg_c, lengths, wt, shared_f):
    """Per-batch tensors: multiplicative mask (transposed layout), scaled
    one-hot pool matrix, addvecT (pos_enc + size_emb gather + folded ob')."""
    pad = (wt - 1) // 2
    # maskM: [P, NT * wt * P]; cols t*(wt*P) + w*P + r; 1.0 where attends
    maskM = np.zeros((P, NT * wt * P), dtype=np.float32)
    for t in range(NT):
        ci = min(max(t - pad, 0), NT - wt)
        seg_r = seg[t * P:(t + 1) * P]            # [128] row segs
        row_ok = seg_r >= 0
        for w in range(wt):
            seg_w = seg[(ci + w) * P:(ci + w + 1) * P]   # [128] win segs
            m = (seg_w[:, None] == seg_r[None, :]) & row_ok[None, :]
            blk = maskM[:, t * wt * P + w * P: t * wt * P + (w + 1) * P]
            blk[m] = 1.0
    # O'r: [P, NT*MAXC]; orm[p, t*MAXC + c] = 1/len_c if seg_c[t*128+p]==c
    orm = np.zeros((P, NT * MAXC), dtype=np.float32)
    recip = (1.0 / np.maximum(lengths, 1)).astype(np.float32)
    for t in range(NT):
        sc = seg_c[t * P:(t + 1) * P]
        okm = sc < MAXC
        orm[np.nonzero(okm)[0], t * MAXC + sc[okm]] = recip[sc[okm]]
    # addvec1 = addvec @ W1.T + b1, via pre-multiplied tables (host gather
    # only): pe1 + (len>0) * (se1[size_idx] + ob1)  [MAXC, 2D]
    ne = (lengths > 0).astype(np.float32)[:, None]
    size_idx = np.minimum(lengths, MAXSEQ - 1)
    addvec1 = (shared_f["pe1"]
               + ne * (shared_f["se1"][size_idx] + shared_f["ob1"][None, :]))
    # addv1T [P, N2*MAXC]: col n*MAXC + c, row = hid within n
    addv1T = np.ascontiguousarray(
        addvec1.T.reshape(N2, P, MAXC).transpose(1, 0, 2)
        .reshape(P, N2 * MAXC).astype(np.float16))
    return maskM.astype(np.float16), orm.astype(np.float16), addv1T


def _pack_weights(in_proj_w, in_proj_b, out_w, out_b, w1, b1, w2, b2,
                  ln_g, ln_b, pos_enc, size_emb):
    f32 = np.float32
    ipw = np.asarray(in_proj_w, f32)
    wqT = ipw[0:D].T            # [din, dout]
    wkT = ipw[D:2 * D].T
    wvT = ipw[2 * D:3 * D].T
    woT = np.asarray(out_w, f32).T
    w1T = np.asarray(w1, f32).T       # [D, 2D]
    w2T = np.asarray(w2, f32).T       # [2D, D]

    def h16(a):
        return np.ascontiguousarray(a.astype(np.float16))

    # wv16 [DD3*P, KD*512]: row dd3*P+p, col kd*512+c
    wv16 = h16(wvT.reshape(KD, P, DD3, 512).transpose(2, 1, 0, 3)
               .reshape(DD3 * P, KD * 512))
    # wqk8 [NG*P, HG*2*KD*P] fp8: row g*P+p,
    # col hl*(2*KD*P) + qk*(KD*P) + dk*(2*P) + half*P + m  (DoubleRow A/B)
    wq8, sq = _q8(wqT)
    wk8, sk = _q8(wkT)
    wq6 = np.asarray(wq8).reshape(DK2, 2, P, NG, HG, P)  # dk half p g hl m
    wk6 = np.asarray(wk8).reshape(DK2, 2, P, NG, HG, P)
    wqk = np.stack([wq6, wk6], axis=0)           # qk dk half p g hl m
    wqk8 = np.ascontiguousarray(
        wqk.transpose(4, 3, 5, 0, 1, 2, 6)       # g p hl qk dk half m
        .reshape(NG * P, HG * 2 * KD * P))
    # fold the out-projection into W1: W1' = w1 @ out_w  [2D, D]
    w1f = np.asarray(w1, f32) @ np.asarray(out_w, f32)
    w1fT = w1f.T                      # [D, 2D]
    # w1216 [P, N2*W12]: per n, [KD*P w1'-lhsT cols | D w2-rhs cols]
    w1p = w1fT.reshape(KD, P, N2, P).transpose(2, 1, 0, 3).reshape(
        N2, P, KD * P)
    w2p = w2T.reshape(N2, P, D)
    w1216 = h16(np.concatenate([w1p, w2p], axis=2)    # n p (KD*P + D)
                .transpose(1, 0, 2).reshape(P, N2 * W12))

    bq = np.asarray(in_proj_b[0:D], f32)
    bk = np.asarray(in_proj_b[D:2 * D], f32)
    bv = np.asarray(in_proj_b[2 * D:3 * D], f32)
    ob = np.asarray(out_b, f32)
    obp = np.asarray(out_w, f32) @ bv + ob       # folded ob' = bv @ Wo^T + ob

    shared = {
        "wv16": wv16, "wqk8": wqk8, "w1216": w1216,
        "sx_ref": np.zeros(1, np.float32),  # filled by prepare_inputs
        "sq": sq, "sk": sk,
        "bqs": np.ascontiguousarray(bq.reshape(H, P).T * INV_SD),
        "bkp": np.ascontiguousarray(bk.reshape(H, P).T),
        "b2row": h16(np.asarray(b2, f32).reshape(1, D)),
        "lngb": np.ascontiguousarray(
            np.broadcast_to(np.asarray(ln_g, f32), (P, D))),
        "lnbb": np.ascontiguousarray(
            np.broadcast_to(np.asarray(ln_b, f32), (P, D))),
    }
    w1Tf = np.asarray(w1, f32).T
    b1f = np.asarray(b1, f32)
    shared_f = {
        "pe1": np.asarray(pos_enc, f32).reshape(MAXC, D) @ w1Tf + b1f,
        "se1": np.asarray(size_emb, f32) @ w1Tf,
        "ob1": obp @ w1Tf,
    }
    return shared, shared_f


# ------------------------------------------------------------- device build

def build_nc(wt, sim_safe=False, repeat=1):
    WB = wt * P
    nc = bacc.Bacc("TRN2", target_bir_lowering=False, debug=False)
    dp = nc.declare_dram_parameter
    dram = {
        "xT16": dp("xT16", [P, KD * S], F16, isOutput=False),
        "xT8": dp("xT8", [P, KD * S], F8, isOutput=False),
        "wv16": dp("wv16", [DD3 * P, KD * 512], F16, isOutput=False),
        "wqk8": dp("wqk8", [NG * P, HG * 2 * KD * P], F8, isOutput=False),
        "qksc": dp("qksc", [P, 2], F32, isOutput=False),
        "w1216": dp("w1216", [P, N2 * W12], F16, isOutput=False),
        "maskM": dp("maskM", [P, NT * WB], F16, isOutput=False),
        "orm": dp("orm", [P, NT * MAXC], F16, isOutput=False),
        "addv1T": dp("addv1T", [P, N2 * MAXC], F16, isOutput=False),
        "bqs": dp("bqs", [P, H], F32, isOutput=False),
        "bkp": dp("bkp", [P, H], F32, isOutput=False),
        "b2row": dp("b2row", [1, D], F16, isOutput=False),
        "lngb": dp("lngb", [P, D], F32, isOutput=False),
        "lnbb": dp("lnbb", [P, D], F32, isOutput=False),
        "out": dp("out", [MAXC, D], F32, isOutput=True),
    }
    with ExitStack() as octx:
        tc = octx.enter_context(tile.TileContext(nc))
        for _rep in range(repeat):
            _emit(nc, tc, wt, sim_safe, dram)
    nc.finalize()
    return nc


def _emit_qk_block(nc, xT8, wqk, qt, kt, bqs, bkp, qksc, psqk, g, j):
    """One QK psum block: j indexes (hl, qk, half); 6 fp8 DoubleRow
    matmuls (K=256 each) + ACT copy (descale + bias)."""
    hl, qk, half = j // 4, (j // 2) % 2, j % 2
    h = g * HG + hl
    dst, bias = (qt, bqs) if qk == 0 else (kt, bkp)
    pq = psqk.tile([P, 512], F32, tag="pq", name=f"pq{g}_{j}")
    for dk in range(DK2):
        wb = wqk[:, hl * 2 * KD * P + qk * KD * P + dk * 2 * P:
                 hl * 2 * KD * P + qk * KD * P + dk * 2 * P + 1]
        xb = xT8[:, dk * 2 * S + half * 512: dk * 2 * S + half * 512 + 1]
        nc.tensor.matmul(
            pq[:],
            _strided_ap(wb, 0, [[P, 2], [1, P]]),
            _strided_ap(xb, 0, [[S, 2], [1, 512]]),
            start=(dk == 0), stop=(dk == DK2 - 1), perf_mode=DR)
    nc.scalar.activation(
        dst[:, hl * S + half * 512: hl * S + (half + 1) * 512],
        pq[:], AF.Identity, bias=bias[:, h:h + 1],
        scale=qksc[:, qk:qk + 1])


def _emit(nc, tc, wt, sim_safe, dram):
    d = dram
    WB = wt * P
    pad = (wt - 1) // 2
    ct_idx = [min(max(t - pad, 0), NT - wt) for t in range(NT)]
    DEPTH = 4  # attention software-pipeline depth (exp/mask latency hiding)

    with ExitStack() as ctx:
        persist = ctx.enter_context(tc.tile_pool(name="persist", bufs=1))

        xp_ctx = ExitStack()
        # pool-open order = SBUF stack order; wvs first so its early-freed
        # range (V-proj done ~25% in) sits where stage-3 tensors land.
        wvs = xp_ctx.enter_context(tc.tile_pool(name="wvs", bufs=2))
        xp = xp_ctx.enter_context(tc.tile_pool(name="xp", bufs=1))
        # DMA priority order: xT (everything waits on it), QK weights +
        # biases for group 0, V weights, then the rest.
        xT8 = xp.tile([P, KD * S], F8, tag="xT8")
        nc.sync.dma_start(xT8[:], d["xT8"].ap()[:])
        bqs = persist.tile([P, H], F32, tag="bqs")
        nc.sync.dma_start(bqs[:], d["bqs"].ap()[:])
        bkp = persist.tile([P, H], F32, tag="bkp")
        nc.sync.dma_start(bkp[:], d["bkp"].ap()[:])
        qksc = persist.tile([P, 2], F32, tag="qksc")
        nc.sync.dma_start(qksc[:], d["qksc"].ap()[:])
        xT = xp.tile([P, KD * S], F16, tag="xT")

        # ============ stage 1: V-proj; pipelined QK-proj + attention =======
        with tc.tile_pool(name="wqks", bufs=2) as wqks, \
             tc.tile_pool(name="qkp", bufs=2) as qkp, \
             tc.tile_pool(name="psqk", bufs=2, space="PSUM") as psqk, \
             tc.tile_pool(name="aw", bufs=DEPTH + 2) as aw, \
             tc.tile_pool(name="smallp", bufs=4) as smallp:

            # weight DMAs: first two groups' QK weights + V weights up
            # front (no WAR waits; SP queue flows); later groups fetched
            # inside the loop once their double-buffer slot frees.
            wqk_t = {}
            HLW = KD * 2 * P

            def fetch_wqk(g, split=False):
                w_ = wqks.tile([P, HG * HLW], F8, tag="wqk",
                               name=f"wqk{g}")
                if split:
                    for hl in range(HG):
                        nc.sync.dma_start(
                            w_[:, hl * HLW:(hl + 1) * HLW],
                            d["wqk8"].ap()[g * P:(g + 1) * P,
                                           hl * HLW:(hl + 1) * HLW])
                else:
                    nc.sync.dma_start(w_[:],
                                      d["wqk8"].ap()[g * P:(g + 1) * P, :])
                wqk_t[g] = w_

            fetch_wqk(0, split=True)
            nc.sync.dma_start(xT[:], d["xT16"].ap()[:])
            wv_t = []
            for dd3 in range(DD3):
                w_ = wvs.tile([P, KD * 512], F16, tag="wv", name=f"wv{dd3}")
                nc.sync.dma_start(w_[:], d["wv16"].ap()[dd3 * P:(dd3 + 1) * P, :])
                wv_t.append(w_)
            fetch_wqk(1)

            # big tensors threaded across stages + small consts
            v16 = persist.tile([P, NT * HB], F16, tag="v16")
            vap = v16[:, 0:1]
            nc.vector.memset(_strided_ap(vap, P, [[VW, NT * H], [1, 1]]), 1.0)
            ctx16 = persist.tile([P, NT * D], F16, tag="ctx16")
            maskM = persist.tile([P, NT * WB], F16, tag="maskM")
            nc.sync.dma_start(maskM[:], d["maskM"].ap()[:])
            ones_row = persist.tile([1, P], F16, tag="ones_row")
            nc.vector.memset(ones_row[:], 1.0)
            b2row = persist.tile([1, D], F16, tag="b2row")
            nc.sync.dma_start(b2row[:], d["b2row"].ap()[:])
            eps_ln = persist.tile([P, 1], F32, tag="eps_ln")
            nc.vector.memset(eps_ln[:], 1e-5)

            qts, kts = {}, {}

            def new_qk(g):
                qts[g] = qkp.tile([P, HG * S], F16, tag="qt", name=f"qt{g}")
                kts[g] = qkp.tile([P, HG * S], F16, tag="kt", name=f"kt{g}")

            # QK-proj of group 0 first (pure PE streak right after xT lands)
            new_qk(0)
            for j in range(4 * HG):
                _emit_qk_block(nc, xT8, wqk_t[0], qts[0], kts[0], bqs, bkp,
                               qksc, psqk, 0, j)

            # V-proj (must complete before attnV of group 0)
            with tc.tile_pool(name="psv", bufs=3, space="PSUM") as psv:
                for dd3 in range(DD3):
                    for mt in range(NT):
                        pv = psv.tile([P, 512], F32, tag="pv")
                        for kd in range(KD):
                            nc.tensor.matmul(
                                pv[:],
                                xT[:, kd * S + mt * P: kd * S + (mt + 1) * P],
                                wv_t[dd3][:, kd * 512:(kd + 1) * 512],
                                start=(kd == 0), stop=(kd == KD - 1))
                        vbase = v16[:, mt * HB + dd3 * 4 * VW:
                                    mt * HB + dd3 * 4 * VW + 1]
                        nc.scalar.activation(
                            _strided_ap(vbase, 0, [[VW, 4], [1, P]]),
                            pv[:].rearrange("p (h m) -> p h m", m=P),
                            AF.Identity)
            atn_ctx = ExitStack()
            pssc = atn_ctx.enter_context(
                tc.tile_pool(name="pssc", bufs=3, space="PSUM"))
            pscx = atn_ctx.enter_context(
                tc.tile_pool(name="pscx", bufs=2, space="PSUM"))

            # attention(g) emission-interleaved with QK-proj(g+1)
            def attn_emit(state):
                """Advance the attention software pipeline by one slot."""
                g = state["g"]
                i = state["i"]
                items = state["items"]
                if i < len(items):
                    t, hl = items[i]
                    ci = ct_idx[t]
                    if hl == 0:
                        state["pc"][t] = pscx.tile(
                            [P, HG * VW], F32, tag="pc", name=f"pc{g}_{t}")
                    sc = pssc.tile([P, WB], F32, tag="sc", name=f"sc{g}_{i}")
                    qt, kt = qts[g], kts[g]
                    for w in range(wt):
                        nc.tensor.matmul(
                            sc[:, w * P:(w + 1) * P],
                            kt[:, hl * S + (ci + w) * P:
                               hl * S + (ci + w + 1) * P],
                            qt[:, hl * S + t * P: hl * S + (t + 1) * P],
                            start=True, stop=True)
                    ex = aw.tile([P, WB], F16, tag="ex", name=f"ex{g}_{i}")
                    nc.scalar.activation(ex[:], sc[:], AF.Exp)
                    nc.vector.tensor_tensor(
                        ex[:], ex[:], maskM[:, t * WB:(t + 1) * WB], ALU.mult)
                    state["ex"][i] = ex
                if i >= DEPTH and i - DEPTH < len(items):
                    t, hl = items[i - DEPTH]
                    h = g * HG + hl
                    ci = ct_idx[t]
                    ex = state["ex"].pop(i - DEPTH)
                    pc = state["pc"][t]
                    for w in range(wt):
                        nc.tensor.matmul(
                            pc[:, hl * VW:(hl + 1) * VW],
                            ex[:, w * P:(w + 1) * P],
                            v16[:, (ci + w) * HB + h * VW:
                                (ci + w) * HB + (h + 1) * VW],
                            start=(w == 0), stop=(w == wt - 1))
                    if hl == HG - 1:
                        # batched max(den,eps) + recip for the 3 heads
                        dmx = smallp.tile([P, HG], F32, tag="dmx",
                                          name=f"dmx{g}_{t}")
                        nc.vector.tensor_scalar_max(
                            dmx[:], _strided_ap(pc[:, 0:1], P, [[VW, HG]]),
                            EPS_DEN)
                        rden = smallp.tile([P, HG], F32, tag="rden",
                                           name=f"rden{g}_{t}")
                        nc.vector.reciprocal(rden[:], dmx[:])
                        for hl2 in range(HG):
                            h2_ = g * HG + hl2
                            nc.vector.tensor_scalar_mul(
                                ctx16[:, t * D + h2_ * P:
                                      t * D + (h2_ + 1) * P],
                                pc[:, hl2 * VW:hl2 * VW + P],
                                rden[:, hl2:hl2 + 1])
                state["i"] += 1

            for g in range(NG):
                if g + 2 < NG:
                    fetch_wqk(g + 2)
                state = {"g": g, "i": 0, "ex": {}, "pc": {},
                         "items": [(t, hl) for t in range(NT)
                                   for hl in range(HG)]}
                n_steps = len(state["items"]) + DEPTH
                if g + 1 < NG:
                    new_qk(g + 1)
                    # interleave: 12 QK blocks across the 24+DEPTH attn slots
                    qk_jobs = list(range(4 * HG))
                    for step in range(n_steps):
                        if step % 2 == 0 and qk_jobs:
                            j = qk_jobs.pop(0)
                            _emit_qk_block(nc, xT8, wqk_t[g + 1], qts[g + 1],
                                           kts[g + 1], bqs, bkp, qksc, psqk,
                                           g + 1, j)
                        attn_emit(state)
                    for j in qk_jobs:
                        _emit_qk_block(nc, xT8, wqk_t[g + 1], qts[g + 1],
                                       kts[g + 1], bqs, bkp, qksc, psqk,
                                       g + 1, j)
                else:
                    for step in range(n_steps):
                        attn_emit(state)
            atn_ctx.close()
        xp_ctx.close()

        # ============ stage 2+3: pooling -> pooledT; fused MLP =============
        # (out-projection is folded into W1 on the host: W1' = W1 @ Wo,
        # addvec1 = addvec @ W1.T + b1)
        with tc.tile_pool(name="w12s", bufs=3) as w12s, \
             tc.tile_pool(name="adp", bufs=1) as adp:
            addv1T = adp.tile([P, N2 * MAXC], F16, tag="addv1T")
            nc.sync.dma_start(addv1T[:], d["addv1T"].ap()[:])
            lngb = persist.tile([P, D], F32, tag="lngb")
            nc.sync.dma_start(lngb[:], d["lngb"].ap()[:])
            lnbb = persist.tile([P, D], F32, tag="lnbb")
            nc.sync.dma_start(lnbb[:], d["lnbb"].ap()[:])
            pooledT = persist.tile([P, KD * MAXC], F16, tag="pooledT")
            orm = persist.tile([P, NT * MAXC], F16, tag="orm")
            nc.sync.dma_start(orm[:], d["orm"].ap()[:])
            w12_t = {}

            def fetch_w12(nch):
                w12_t[nch] = w12s.tile([P, MCH * W12], F16, tag="w12",
                                       name=f"w12c{nch}")
                nc.sync.dma_start(
                    w12_t[nch][:],
                    d["w1216"].ap()[:, nch * MCH * W12:(nch + 1) * MCH * W12])

            fetch_w12(0)
            fetch_w12(1)
            fetch_w12(2)
            with tc.tile_pool(name="pspool", bufs=2, space="PSUM") as pspool:
                for fd in range(KD):
                    pp = pspool.tile([P, MAXC], F32, tag="pp")
                    for t in range(NT):
                        nc.tensor.matmul(
                            pp[:],
                            ctx16[:, t * D + fd * P: t * D + (fd + 1) * P],
                            orm[:, t * MAXC:(t + 1) * MAXC],
                            start=(t == 0), stop=(t == NT - 1))
                    nc.scalar.activation(
                        pooledT[:, fd * MAXC:(fd + 1) * MAXC], pp[:],
                        AF.Identity)

            # ======== stage 4+5: fused MLP h1(n)/h2(n-1) pipeline ==========
            with tc.tile_pool(name="h1s", bufs=N2 + 1) as h1s, \
                 tc.tile_pool(name="hadd", bufs=3) as hadd, \
                 tc.tile_pool(name="lnp", bufs=2) as lnp, \
                 tc.tile_pool(name="lnq", bufs=1) as lnq, \
                 tc.tile_pool(name="lns", bufs=6) as lns, \
                 tc.tile_pool(name="psh1", bufs=2, space="PSUM") as psh1, \
                 tc.tile_pool(name="psh2", bufs=1, space="PSUM") as psh2:
                h2ps = [[psh2.tile([P, 512], F32, tag=f"ph2_{c}_{j}",
                                   name=f"ph2_{c}_{j}")
                         for j in range(DD3)] for c in range(CT)]
                h1n = {}

                def emit_h1(n):
                    nch, nl = n // MCH, n % MCH
                    ph = psh1.tile([P, MAXC], F32, tag="ph", name=f"ph{n}")
                    wchunk = w12_t[nch]
                    for kd in range(KD):
                        nc.tensor.matmul(
                            ph[:],
                            wchunk[:, nl * W12 + kd * P:
                                   nl * W12 + (kd + 1) * P],
                            pooledT[:, kd * MAXC:(kd + 1) * MAXC],
                            start=(kd == 0), stop=(kd == KD - 1))
                    ha = hadd.tile([P, MAXC], F16, tag="hadd", name=f"ha{n}")
                    nc.vector.tensor_tensor(
                        ha[:], ph[:], addv1T[:, n * MAXC:(n + 1) * MAXC],
                        ALU.add)
                    h1n[n] = h1s.tile([P, MAXC], F16, tag="h1n",
                                      name=f"h1n{n}")
                    nc.scalar.activation(
                        h1n[n][:], ha[:],
                        AF.Identity if sim_safe else AF.Gelu)

                def emit_h2(n, c):
                    nch, nl = n // MCH, n % MCH
                    wchunk = w12_t[nch]
                    src = h1n[n]
                    for dd3 in range(DD3):
                        nc.tensor.matmul(
                            h2ps[c][dd3][:],
                            src[:, c * P:(c + 1) * P],
                            wchunk[:, nl * W12 + KD * P + dd3 * 512:
                                   nl * W12 + KD * P + (dd3 + 1) * 512],
                            start=(n == 0), stop=False)

                # c=0's h2 accumulates inline with h1; c=1's h2 runs after,
                # overlapping c=0's LayerNorm tail on DVE/ACT.
                for n in range(N2 + 1):
                    if n < N2:
                        if n % MCH == 0 and n // MCH + 3 < N2 // MCH:
                            fetch_w12(n // MCH + 3)
                        emit_h1(n)
                    if n >= 1:
                        emit_h2(n - 1, 0)

                def emit_c1_h2():
                    for n in range(N2):
                        emit_h2(n, 1)

                # ---- b2 + LayerNorm (c=0's LN overlaps c=1's h2 matmuls)
                def ln_c(c):
                    h2 = lnp.tile([P, D], F32, tag="h2", name=f"h2_{c}")
                    parts = []
                    for dd3 in range(DD3):
                        nc.tensor.matmul(
                            h2ps[c][dd3][:], ones_row[:],
                            b2row[:, dd3 * 512:(dd3 + 1) * 512],
                            start=False, stop=True)
                        pacc = lns.tile([P, 1], F32, tag="pacc",
                                        name=f"pacc{c}_{dd3}")
                        nc.scalar.activation(
                            h2[:, dd3 * 512:(dd3 + 1) * 512], h2ps[c][dd3][:],
                            AF.Identity, accum_out=pacc[:])
                        parts.append(pacc)
                    s01 = lns.tile([P, 1], F32, tag="s01", name=f"s01_{c}")
                    nc.vector.tensor_tensor(s01[:], parts[0][:], parts[1][:],
                                            ALU.add)
                    s012 = lns.tile([P, 1], F32, tag="s012", name=f"s012_{c}")
                    nc.vector.tensor_tensor(s012[:], s01[:], parts[2][:],
                                            ALU.add)
                    negmu = lns.tile([P, 1], F32, tag="negmu")
                    nc.vector.tensor_scalar_mul(negmu[:], s012[:], -1.0 / D)
                    ssq = lns.tile([P, 1], F32, tag="ssq")
                    sq = lnq.tile([P, D], F32, tag="sq", name=f"sq_{c}")
                    nc.scalar.activation(sq[:], h2[:], AF.Square,
                                         bias=negmu[:], accum_out=ssq[:])
                    std = lns.tile([P, 1], F32, tag="std")
                    nc.scalar.activation(std[:], ssq[:], AF.Sqrt,
                                         bias=eps_ln[:], scale=1.0 / D)
                    rstd = lns.tile([P, 1], F32, tag="rstd")
                    nc.vector.reciprocal(rstd[:], std[:])
                    t1 = lnp.tile([P, D], F32, tag="t1", name=f"t1_{c}")
                    for dd3 in range(DD3):
                        sl = slice(dd3 * 512, (dd3 + 1) * 512)
                        eng = nc.vector if dd3 != 1 else nc.gpsimd
                        nc.vector.tensor_scalar(t1[:, sl], h2[:, sl],
                                                negmu[:], rstd[:],
                                                ALU.add, ALU.mult)
                        eng.tensor_tensor(t1[:, sl], t1[:, sl], lngb[:, sl],
                                          ALU.mult)
                        eng.tensor_tensor(t1[:, sl], t1[:, sl], lnbb[:, sl],
                                          ALU.add)
                        nc.scalar.dma_start(
                            d["out"].ap()[c * P:(c + 1) * P, sl], t1[:, sl])

                ln_c(0)
                emit_c1_h2()
                ln_c(1)


# ------------------------------------------------------------------ driver

def prepare_inputs(x, boundaries, in_proj_w, in_proj_b, out_w, out_b,
                   w1, b1, w2, b2, ln_g, ln_b, pos_enc, size_emb):
    """Host prep: returns (wt, in_maps) for the 8 cores."""
    x = np.asarray(x, dtype=np.float32)
    boundaries = np.asarray(boundaries, dtype=np.float32)
    segs = [_host_segments(boundaries[b]) for b in range(B)]
    wt = _window_tiles([s[0] for s in segs])

    shared, shared_f = _pack_weights(in_proj_w, in_proj_b, out_w, out_b,
                                     w1, b1, w2, b2, ln_g, ln_b,
                                     pos_enc, size_emb)
    in_maps = []
    for b in range(B):
        seg, valid, seg_c, lengths = segs[b]
        maskM, orm, addv1T = _host_per_batch(seg, valid, seg_c, lengths,
                                             wt, shared_f)
        m = dict(shared)
        # xT16 [P, KD*S]: row p, col kd*S + tok
        xTb = np.ascontiguousarray(
            x[b].T.reshape(KD, P, S).transpose(1, 0, 2).reshape(P, KD * S))
        m["xT16"] = xTb.astype(np.float16)
        x8, sx = _q8(xTb)
        m["xT8"] = x8
        m["qksc"] = np.broadcast_to(np.asarray(
            [INV_SD / (sx * shared["sq"]), 1.0 / (sx * shared["sk"])],
            np.float32), (P, 2)).copy()
        del m["sx_ref"], m["sq"], m["sk"]
        m["maskM"] = maskM
        m["orm"] = orm
        m["addv1T"] = addv1T
        in_maps.append(m)
    return wt, in_maps


_NC_CACHE = {}


def get_nc(wt):
    if wt not in _NC_CACHE:
        _NC_CACHE[wt] = build_nc(wt)
    return _NC_CACHE[wt]


def kernel(**inputs):
    wt, in_maps = prepare_inputs(**inputs)
    nc = get_nc(wt)
    res = run_bass_kernel_spmd(nc, in_maps, list(range(B)))
    out = np.stack([res.results[b]["out"] for b in range(B)], axis=0)
    return out.astype(np.float32)


# revision 27
# speedup vs baseline: 1.3435x; 1.3435x over previous
"""Trainium2 Bass kernel for nn_EnhancedChunkLayer (ragged_sequence).

Strategy: data-parallel over batch B=8 across 8 NeuronCores (one batch
element per core, weights replicated). Inside each core (all PE matmul
operands fp16; fp32 PSUM accumulation, softmax and LayerNorm math):

  - banded block-diagonal attention with a centered wt*128-wide column
    window; scores are computed TRANSPOSED (scoresT[wincol, row] =
    K_h-layout @ Q_h) so the exp'd probs feed attn@V directly as the
    stationary operand -- no PE transposes at all.
  - softmax: exp directly off the scores PSUM (ACT), then a MULTIPLICATIVE
    {0,1} fp16 mask on DVE (exp(s+m) == exp(s)*exp(m)); V carries an
    interleaved 129th ones-column per head so the denominator accumulates
    inside the attn@V matmul itself; per-(t) batched max(den,eps) +
    reciprocal on DVE via a strided AP over the den columns; the per-row
    1/den folds into the per-head ctx PSUM->SBUF copy (Pool engine,
    per-partition scalar).
  - QK-projection of head-group g+1 is emission-interleaved with the
    attention of group g, so the in-order PE always has projection matmuls
    to chew on while ACT/DVE run the softmax chain.
  - mean-pool FIRST (on ctx, via a host-built recip-scaled one-hot matmul),
    THEN the out-projection on [MAXC, D] instead of [S, D] (4x fewer rows).
    bv folds into ob' = bv @ Wo^T + ob on the host; ob', the size-embedding
    lookup (host gather) and pos_enc fold into one addvecT tensor.
  - chunk MLP streams W1/W2 in combined per-n chunks with h1(n) / h2(n-1)
    software-pipelined; exact-erf GELU on ACT; LayerNorm on chip.

Host does index bookkeeping only: cumsum of boundary indicators, mask /
scaled-one-hot construction, the size-embedding row gather, bias folds, and
weight layout packs (fp16).
"""

import math
from contextlib import ExitStack

import numpy as np

import concourse.bacc as bacc
import concourse.bass as bass
import concourse.mybir as mybir
from concourse import tile
from concourse.bass_utils import run_bass_kernel_spmd

F32 = mybir.dt.float32
F16 = mybir.dt.float16
F8 = mybir.dt.float8e4
DR = mybir.MatmulPerfMode.DoubleRow
AF = mybir.ActivationFunctionType
ALU = mybir.AluOpType
AX = mybir.AxisListType

B, S, D = 8, 1024, 1536
H, DH = 12, 128
MAXC, MAXSEQ = 256, 1024
THRESH = 0.85
P = 128
KD = D // P          # 12 contraction tiles over D
NT = S // P          # 8 row tiles over S
N2 = (2 * D) // P    # 24 tiles over hidden 2D
CT = MAXC // P       # 2 chunk tiles
DD3 = D // 512       # 3 free-dim 512 tiles over D
HG = 3               # heads per group
NG = H // HG         # 4 head groups
VW = 129             # per-head V block: 128 feats + 1 ones col (den)
HB = H * VW          # per-tile V block width
W12 = KD * P + D     # combined per-n W1 col block + W2 row block
MCH = 2              # n-tiles per streamed MLP chunk
DK2 = KD // 2        # 6 double-contraction tiles (K=256, fp8 DoubleRow)
INV_SD = 1.0 / math.sqrt(DH)
EPS_DEN = 2.0 ** -14  # fp16 min normal; keeps fully-masked rows finite


def _q8(a, target=160.0):
    """Quantize to e4m3 with a power-of-2 scale; returns (quant, scale)."""
    a = np.asarray(a, np.float32)
    m = float(np.max(np.abs(a))) or 1.0
    sc = 2.0 ** np.floor(np.log2(target / m))
    return np.ascontiguousarray((a * sc).astype(mybir.dt.np(F8))), sc


def _strided_ap(base, extra_off, dims):
    """AP over `base`'s tensor at base.offset+extra_off with free dims
    `dims` (list of [stride, count]); partition dim copied from base."""
    return bass.AP(tensor=base.tensor, offset=base.offset + extra_off,
                   ap=[list(base.ap[0])] + [list(d) for d in dims])


# ---------------------------------------------------------------- host prep

def _host_segments(boundaries_b):
    is_b = boundaries_b > THRESH
    seg = np.cumsum(is_b.astype(np.int64)) - 1
    valid = seg >= 0
    seg_c = np.where(valid & (seg < MAXC), seg, MAXC)
    lengths = np.bincount(seg_c, minlength=MAXC + 1)[:MAXC]
    return seg, valid, seg_c, lengths


def _window_tiles(seg_list):
    """Smallest odd tile-count window covering every chunk from any row tile."""
    wt = 3
    while True:
        if wt > NT:
            return NT
        pad = (wt - 1) // 2
        ok = True
        for seg in seg_list:
            for t in range(NT):
                ci = min(max(t - pad, 0), NT - wt)
                lo, hi = ci * P, ci * P + wt * P
                rows = np.arange(t * P, (t + 1) * P)
                segs = seg[rows]
                vmask = segs >= 0
                if not vmask.any():
                    continue
                cols = np.isin(seg, segs[vmask]) & (seg >= 0)
                idx = np.nonzero(cols)[0]
                if len(idx) and (idx[0] < lo or idx[-1] >= hi):
                    ok = False
                    break
            if not ok:
                break
        if ok:
            return wt
        wt += 2


def _host_per_batch(seg, valid, seg_c, lengths, wt, shared_f):
    """Per-batch tensors: multiplicative mask (transposed layout), scaled
    one-hot pool matrix, addvecT (pos_enc + size_emb gather + folded ob')."""
    pad = (wt - 1) // 2
    # maskM: [P, NT * wt * P]; cols t*(wt*P) + w*P + r; 1.0 where attends
    maskM = np.zeros((P, NT * wt * P), dtype=np.float32)
    for t in range(NT):
        ci = min(max(t - pad, 0), NT - wt)
        seg_r = seg[t * P:(t + 1) * P]            # [128] row segs
        row_ok = seg_r >= 0
        for w in range(wt):
            seg_w = seg[(ci + w) * P:(ci + w + 1) * P]   # [128] win segs
            m = (seg_w[:, None] == seg_r[None, :]) & row_ok[None, :]
            blk = maskM[:, t * wt * P + w * P: t * wt * P + (w + 1) * P]
            blk[m] = 1.0
    # O'r: [P, NT*MAXC]; orm[p, t*MAXC + c] = 1/len_c if seg_c[t*128+p]==c
    orm = np.zeros((P, NT * MAXC), dtype=np.float32)
    recip = (1.0 / np.maximum(lengths, 1)).astype(np.float32)
    for t in range(NT):
        sc = seg_c[t * P:(t + 1) * P]
        okm = sc < MAXC
        orm[np.nonzero(okm)[0], t * MAXC + sc[okm]] = recip[sc[okm]]
    # addvec1 = addvec @ W1.T + b1, via pre-multiplied tables (host gather
    # only): pe1 + (len>0) * (se1[size_idx] + ob1)  [MAXC, 2D]
    ne = (lengths > 0).astype(np.float32)[:, None]
    size_idx = np.minimum(lengths, MAXSEQ - 1)
    addvec1 = (shared_f["pe1"]
               + ne * (shared_f["se1"][size_idx] + shared_f["ob1"][None, :]))
    # addv1T [P, N2*MAXC]: col n*MAXC + c, row = hid within n
    addv1T = np.ascontiguousarray(
        addvec1.T.reshape(N2, P, MAXC).transpose(1, 0, 2)
        .reshape(P, N2 * MAXC).astype(np.float16))
    return maskM.astype(np.float16), orm.astype(np.float16), addv1T


def _pack_weights(in_proj_w, in_proj_b, out_w, out_b, w1, b1, w2, b2,
                  ln_g, ln_b, pos_enc, size_emb):
    f32 = np.float32
    ipw = np.asarray(in_proj_w, f32)
    wqT = ipw[0:D].T            # [din, dout]
    wkT = ipw[D:2 * D].T
    wvT = ipw[2 * D:3 * D].T
    woT = np.asarray(out_w, f32).T
    w1T = np.asarray(w1, f32).T       # [D, 2D]
    w2T = np.asarray(w2, f32).T       # [2D, D]

    def h16(a):
        return np.ascontiguousarray(a.astype(np.float16))

    # wv16 [DD3*P, KD*512]: row dd3*P+p, col kd*512+c
    wv16 = h16(wvT.reshape(KD, P, DD3, 512).transpose(2, 1, 0, 3)
               .reshape(DD3 * P, KD * 512))
    # wqk8 [NG*P, HG*2*KD*P] fp8: row g*P+p,
    # col hl*(2*KD*P) + qk*(KD*P) + dk*(2*P) + half*P + m  (DoubleRow A/B)
    wq8, sq = _q8(wqT)
    wk8, sk = _q8(wkT)
    wq6 = np.asarray(wq8).reshape(DK2, 2, P, NG, HG, P)  # dk half p g hl m
    wk6 = np.asarray(wk8).reshape(DK2, 2, P, NG, HG, P)
    wqk = np.stack([wq6, wk6], axis=0)           # qk dk half p g hl m
    wqk8 = np.ascontiguousarray(
        wqk.transpose(4, 3, 5, 0, 1, 2, 6)       # g p hl qk dk half m
        .reshape(NG * P, HG * 2 * KD * P))
    # fold the out-projection into W1: W1' = w1 @ out_w  [2D, D]
    w1f = np.asarray(w1, f32) @ np.asarray(out_w, f32)
    w1fT = w1f.T                      # [D, 2D]
    # w1216 [P, N2*W12]: per n, [KD*P w1'-lhsT cols | D w2-rhs cols]
    w1p = w1fT.reshape(KD, P, N2, P).transpose(2, 1, 0, 3).reshape(
        N2, P, KD * P)
    w2p = w2T.reshape(N2, P, D)
    w1216 = h16(np.concatenate([w1p, w2p], axis=2)    # n p (KD*P + D)
                .transpose(1, 0, 2).reshape(P, N2 * W12))

    bq = np.asarray(in_proj_b[0:D], f32)
    bk = np.asarray(in_proj_b[D:2 * D], f32)
    bv = np.asarray(in_proj_b[2 * D:3 * D], f32)
    ob = np.asarray(out_b, f32)
    obp = np.asarray(out_w, f32) @ bv + ob       # folded ob' = bv @ Wo^T + ob

    shared = {
        "wv16": wv16, "wqk8": wqk8, "w1216": w1216,
        "sx_ref": np.zeros(1, np.float32),  # filled by prepare_inputs
        "sq": sq, "sk": sk,
        "bqs": np.ascontiguousarray(bq.reshape(H, P).T * INV_SD),
        "bkp": np.ascontiguousarray(bk.reshape(H, P).T),
        "b2row": h16(np.asarray(b2, f32).reshape(1, D)),
        "lngb": np.ascontiguousarray(
            np.broadcast_to(np.asarray(ln_g, f32), (P, D))),
        "lnbb": np.ascontiguousarray(
            np.broadcast_to(np.asarray(ln_b, f32), (P, D))),
    }
    w1Tf = np.asarray(w1, f32).T
    b1f = np.asarray(b1, f32)
    shared_f = {
        "pe1": np.asarray(pos_enc, f32).reshape(MAXC, D) @ w1Tf + b1f,
        "se1": np.asarray(size_emb, f32) @ w1Tf,
        "ob1": obp @ w1Tf,
    }
    return shared, shared_f


# ------------------------------------------------------------- device build

def build_nc(wt, sim_safe=False, repeat=1):
    WB = wt * P
    nc = bacc.Bacc("TRN2", target_bir_lowering=False, debug=False)
    dp = nc.declare_dram_parameter
    dram = {
        "xT16": dp("xT16", [P, KD * S], F16, isOutput=False),
        "xT8": dp("xT8", [P, KD * S], F8, isOutput=False),
        "wv16": dp("wv16", [DD3 * P, KD * 512], F16, isOutput=False),
        "wqk8": dp("wqk8", [NG * P, HG * 2 * KD * P], F8, isOutput=False),
        "qksc": dp("qksc", [P, 2], F32, isOutput=False),
        "w1216": dp("w1216", [P, N2 * W12], F16, isOutput=False),
        "maskM": dp("maskM", [P, NT * WB], F16, isOutput=False),
        "orm": dp("orm", [P, NT * MAXC], F16, isOutput=False),
        "addv1T": dp("addv1T", [P, N2 * MAXC], F16, isOutput=False),
        "bqs": dp("bqs", [P, H], F32, isOutput=False),
        "bkp": dp("bkp", [P, H], F32, isOutput=False),
        "b2row": dp("b2row", [1, D], F16, isOutput=False),
        "lngb": dp("lngb", [P, D], F32, isOutput=False),
        "lnbb": dp("lnbb", [P, D], F32, isOutput=False),
        "out": dp("out", [MAXC, D], F32, isOutput=True),
    }
    with ExitStack() as octx:
        tc = octx.enter_context(tile.TileContext(nc))
        for _rep in range(repeat):
            _emit(nc, tc, wt, sim_safe, dram)
    nc.finalize()
    return nc


def _emit_qk_block(nc, xT8, wqk, qt, kt, bqs, bkp, qksc, psqk, g, j):
    """One QK psum block: j indexes (hl, qk, half); 6 fp8 DoubleRow
    matmuls (K=256 each) + ACT copy (descale + bias)."""
    hl, qk, half = j // 4, (j // 2) % 2, j % 2
    h = g * HG + hl
    dst, bias = (qt, bqs) if qk == 0 else (kt, bkp)
    pq = psqk.tile([P, 512], F32, tag="pq", name=f"pq{g}_{j}")
    for dk in range(DK2):
        wb = wqk[:, hl * 2 * KD * P + qk * KD * P + dk * 2 * P:
                 hl * 2 * KD * P + qk * KD * P + dk * 2 * P + 1]
        xb = xT8[:, dk * 2 * S + half * 512: dk * 2 * S + half * 512 + 1]
        nc.tensor.matmul(
            pq[:],
            _strided_ap(wb, 0, [[P, 2], [1, P]]),
            _strided_ap(xb, 0, [[S, 2], [1, 512]]),
            start=(dk == 0), stop=(dk == DK2 - 1), perf_mode=DR)
    nc.scalar.activation(
        dst[:, hl * S + half * 512: hl * S + (half + 1) * 512],
        pq[:], AF.Identity, bias=bias[:, h:h + 1],
        scale=qksc[:, qk:qk + 1])


def _emit(nc, tc, wt, sim_safe, dram):
    d = dram
    WB = wt * P
    pad = (wt - 1) // 2
    ct_idx = [min(max(t - pad, 0), NT - wt) for t in range(NT)]
    DEPTH = 4  # attention software-pipeline depth (exp/mask latency hiding)

    with ExitStack() as ctx:
        persist = ctx.enter_context(tc.tile_pool(name="persist", bufs=1))

        xp_ctx = ExitStack()
        # pool-open order = SBUF stack order; wvs first so its early-freed
        # range (V-proj done ~25% in) sits where stage-3 tensors land.
        wvs = xp_ctx.enter_context(tc.tile_pool(name="wvs", bufs=2))
        xp = xp_ctx.enter_context(tc.tile_pool(name="xp", bufs=1))
        # DMA priority order: xT (everything waits on it), QK weights +
        # biases for group 0, V weights, then the rest.
        xT8 = xp.tile([P, KD * S], F8, tag="xT8")
        nc.sync.dma_start(xT8[:], d["xT8"].ap()[:])
        bqs = persist.tile([P, H], F32, tag="bqs")
        bkp = persist.tile([P, H], F32, tag="bkp")
        qksc = persist.tile([P, 2], F32, tag="qksc")
        xT = xp.tile([P, KD * S], F16, tag="xT")

        # ============ stage 1: V-proj; pipelined QK-proj + attention =======
        with tc.tile_pool(name="wqks", bufs=2) as wqks, \
             tc.tile_pool(name="qkp", bufs=2) as qkp, \
             tc.tile_pool(name="psqk", bufs=2, space="PSUM") as psqk, \
             tc.tile_pool(name="aw", bufs=DEPTH + 2) as aw, \
             tc.tile_pool(name="smallp", bufs=4) as smallp:

            # weight DMAs: first two groups' QK weights + V weights up
            # front (no WAR waits; SP queue flows); later groups fetched
            # inside the loop once their double-buffer slot frees.
            wqk_t = {}
            HLW = KD * 2 * P

            def fetch_wqk(g, split=False):
                w_ = wqks.tile([P, HG * HLW], F8, tag="wqk",
                               name=f"wqk{g}")
                if split:
                    for hl in range(HG):
                        nc.sync.dma_start(
                            w_[:, hl * HLW:(hl + 1) * HLW],
                            d["wqk8"].ap()[g * P:(g + 1) * P,
                                           hl * HLW:(hl + 1) * HLW])
                else:
                    nc.sync.dma_start(w_[:],
                                      d["wqk8"].ap()[g * P:(g + 1) * P, :])
                wqk_t[g] = w_

            fetch_wqk(0, split=True)
            nc.sync.dma_start(bqs[:], d["bqs"].ap()[:])
            nc.sync.dma_start(bkp[:], d["bkp"].ap()[:])
            nc.sync.dma_start(qksc[:], d["qksc"].ap()[:])
            nc.sync.dma_start(xT[:], d["xT16"].ap()[:])
            wv_t = []
            for dd3 in range(DD3):
                w_ = wvs.tile([P, KD * 512], F16, tag="wv", name=f"wv{dd3}")
                nc.sync.dma_start(w_[:], d["wv16"].ap()[dd3 * P:(dd3 + 1) * P, :])
                wv_t.append(w_)
            fetch_wqk(1)

            # big tensors threaded across stages + small consts
            v16 = persist.tile([P, NT * HB], F16, tag="v16")
            vap = v16[:, 0:1]
            nc.vector.memset(_strided_ap(vap, P, [[VW, NT * H], [1, 1]]), 1.0)
            ctx16 = persist.tile([P, NT * D], F16, tag="ctx16")
            maskM = persist.tile([P, NT * WB], F16, tag="maskM")
            nc.sync.dma_start(maskM[:], d["maskM"].ap()[:])
            ones_row = persist.tile([1, P], F16, tag="ones_row")
            nc.vector.memset(ones_row[:], 1.0)
            b2row = persist.tile([1, D], F16, tag="b2row")
            nc.sync.dma_start(b2row[:], d["b2row"].ap()[:])
            eps_ln = persist.tile([P, 1], F32, tag="eps_ln")
            nc.vector.memset(eps_ln[:], 1e-5)

            qts, kts = {}, {}

            def new_qk(g):
                qts[g] = qkp.tile([P, HG * S], F16, tag="qt", name=f"qt{g}")
                kts[g] = qkp.tile([P, HG * S], F16, tag="kt", name=f"kt{g}")

            # QK-proj of group 0 first (pure PE streak right after xT lands)
            new_qk(0)
            for j in range(4 * HG):
                _emit_qk_block(nc, xT8, wqk_t[0], qts[0], kts[0], bqs, bkp,
                               qksc, psqk, 0, j)

            # V-proj (must complete before attnV of group 0)
            with tc.tile_pool(name="psv", bufs=3, space="PSUM") as psv:
                for dd3 in range(DD3):
                    for mt in range(NT):
                        pv = psv.tile([P, 512], F32, tag="pv")
                        for kd in range(KD):
                            nc.tensor.matmul(
                                pv[:],
                                xT[:, kd * S + mt * P: kd * S + (mt + 1) * P],
                                wv_t[dd3][:, kd * 512:(kd + 1) * 512],
                                start=(kd == 0), stop=(kd == KD - 1))
                        vbase = v16[:, mt * HB + dd3 * 4 * VW:
                                    mt * HB + dd3 * 4 * VW + 1]
                        nc.scalar.activation(
                            _strided_ap(vbase, 0, [[VW, 4], [1, P]]),
                            pv[:].rearrange("p (h m) -> p h m", m=P),
                            AF.Identity)
            atn_ctx = ExitStack()
            pssc = atn_ctx.enter_context(
                tc.tile_pool(name="pssc", bufs=3, space="PSUM"))
            pscx = atn_ctx.enter_context(
                tc.tile_pool(name="pscx", bufs=2, space="PSUM"))

            # attention(g) emission-interleaved with QK-proj(g+1)
            def attn_emit(state):
                """Advance the attention software pipeline by one slot."""
                g = state["g"]
                i = state["i"]
                items = state["items"]
                if i < len(items):
                    t, hl = items[i]
                    ci = ct_idx[t]
                    if hl == 0:
                        state["pc"][t] = pscx.tile(
                            [P, HG * VW], F32, tag="pc", name=f"pc{g}_{t}")
                    sc = pssc.tile([P, WB], F32, tag="sc", name=f"sc{g}_{i}")
                    qt, kt = qts[g], kts[g]
                    for w in range(wt):
                        nc.tensor.matmul(
                            sc[:, w * P:(w + 1) * P],
                            kt[:, hl * S + (ci + w) * P:
                               hl * S + (ci + w + 1) * P],
                            qt[:, hl * S + t * P: hl * S + (t + 1) * P],
                            start=True, stop=True)
                    ex = aw.tile([P, WB], F16, tag="ex", name=f"ex{g}_{i}")
                    nc.scalar.activation(ex[:], sc[:], AF.Exp)
                    nc.vector.tensor_tensor(
                        ex[:], ex[:], maskM[:, t * WB:(t + 1) * WB], ALU.mult)
                    state["ex"][i] = ex
                if i >= DEPTH and i - DEPTH < len(items):
                    t, hl = items[i - DEPTH]
                    h = g * HG + hl
                    ci = ct_idx[t]
                    ex = state["ex"].pop(i - DEPTH)
                    pc = state["pc"][t]
                    for w in range(wt):
                        nc.tensor.matmul(
                            pc[:, hl * VW:(hl + 1) * VW],
                            ex[:, w * P:(w + 1) * P],
                            v16[:, (ci + w) * HB + h * VW:
                                (ci + w) * HB + (h + 1) * VW],
                            start=(w == 0), stop=(w == wt - 1))
                    if hl == HG - 1:
                        # batched max(den,eps) + recip for the 3 heads
                        dmx = smallp.tile([P, HG], F32, tag="dmx",
                                          name=f"dmx{g}_{t}")
                        nc.vector.tensor_scalar_max(
                            dmx[:], _strided_ap(pc[:, 0:1], P, [[VW, HG]]),
                            EPS_DEN)
                        rden = smallp.tile([P, HG], F32, tag="rden",
                                           name=f"rden{g}_{t}")
                        nc.vector.reciprocal(rden[:], dmx[:])
                        for hl2 in range(HG):
                            h2_ = g * HG + hl2
                            nc.vector.tensor_scalar_mul(
                                ctx16[:, t * D + h2_ * P:
                                      t * D + (h2_ + 1) * P],
                                pc[:, hl2 * VW:hl2 * VW + P],
                                rden[:, hl2:hl2 + 1])
                state["i"] += 1

            for g in range(NG):
                if g + 2 < NG:
                    fetch_wqk(g + 2)
                state = {"g": g, "i": 0, "ex": {}, "pc": {},
                         "items": [(t, hl) for t in range(NT)
                                   for hl in range(HG)]}
                n_steps = len(state["items"]) + DEPTH
                if g + 1 < NG:
                    new_qk(g + 1)
                    # interleave: 12 QK blocks across the 24+DEPTH attn slots
                    qk_jobs = list(range(4 * HG))
                    for step in range(n_steps):
                        if step % 2 == 0 and qk_jobs:
                            j = qk_jobs.pop(0)
                            _emit_qk_block(nc, xT8, wqk_t[g + 1], qts[g + 1],
                                           kts[g + 1], bqs, bkp, qksc, psqk,
                                           g + 1, j)
                        attn_emit(state)
                    for j in qk_jobs:
                        _emit_qk_block(nc, xT8, wqk_t[g + 1], qts[g + 1],
                                       kts[g + 1], bqs, bkp, qksc, psqk,
                                       g + 1, j)
                else:
                    for step in range(n_steps):
                        attn_emit(state)
            atn_ctx.close()
        xp_ctx.close()

        # ============ stage 2+3: pooling -> pooledT; fused MLP =============
        # (out-projection is folded into W1 on the host: W1' = W1 @ Wo,
        # addvec1 = addvec @ W1.T + b1)
        with tc.tile_pool(name="w12s", bufs=3) as w12s, \
             tc.tile_pool(name="adp", bufs=1) as adp:
            addv1T = adp.tile([P, N2 * MAXC], F16, tag="addv1T")
            nc.sync.dma_start(addv1T[:], d["addv1T"].ap()[:])
            lngb = persist.tile([P, D], F32, tag="lngb")
            nc.sync.dma_start(lngb[:], d["lngb"].ap()[:])
            lnbb = persist.tile([P, D], F32, tag="lnbb")
            nc.sync.dma_start(lnbb[:], d["lnbb"].ap()[:])
            pooledT = persist.tile([P, KD * MAXC], F16, tag="pooledT")
            orm = persist.tile([P, NT * MAXC], F16, tag="orm")
            nc.sync.dma_start(orm[:], d["orm"].ap()[:])
            w12_t = {}

            def fetch_w12(nch):
                w12_t[nch] = w12s.tile([P, MCH * W12], F16, tag="w12",
                                       name=f"w12c{nch}")
                nc.sync.dma_start(
                    w12_t[nch][:],
                    d["w1216"].ap()[:, nch * MCH * W12:(nch + 1) * MCH * W12])

            fetch_w12(0)
            fetch_w12(1)
            fetch_w12(2)
            with tc.tile_pool(name="pspool", bufs=2, space="PSUM") as pspool:
                for fd in range(KD):
                    pp = pspool.tile([P, MAXC], F32, tag="pp")
                    for t in range(NT):
                        nc.tensor.matmul(
                            pp[:],
                            ctx16[:, t * D + fd * P: t * D + (fd + 1) * P],
                            orm[:, t * MAXC:(t + 1) * MAXC],
                            start=(t == 0), stop=(t == NT - 1))
                    nc.scalar.activation(
                        pooledT[:, fd * MAXC:(fd + 1) * MAXC], pp[:],
                        AF.Identity)

            # ======== stage 4+5: fused MLP h1(n)/h2(n-1) pipeline ==========
            with tc.tile_pool(name="h1s", bufs=3) as h1s, \
                 tc.tile_pool(name="hadd", bufs=3) as hadd, \
                 tc.tile_pool(name="lnp", bufs=2) as lnp, \
                 tc.tile_pool(name="lnq", bufs=1) as lnq, \
                 tc.tile_pool(name="lns", bufs=6) as lns, \
                 tc.tile_pool(name="psh1", bufs=2, space="PSUM") as psh1, \
                 tc.tile_pool(name="psh2", bufs=1, space="PSUM") as psh2:
                h2ps = [[psh2.tile([P, 512], F32, tag=f"ph2_{c}_{j}",
                                   name=f"ph2_{c}_{j}")
                         for j in range(DD3)] for c in range(CT)]
                h1n = {}

                def emit_h1(n):
                    nch, nl = n // MCH, n % MCH
                    ph = psh1.tile([P, MAXC], F32, tag="ph", name=f"ph{n}")
                    wchunk = w12_t[nch]
                    for kd in range(KD):
                        nc.tensor.matmul(
                            ph[:],
                            wchunk[:, nl * W12 + kd * P:
                                   nl * W12 + (kd + 1) * P],
                            pooledT[:, kd * MAXC:(kd + 1) * MAXC],
                            start=(kd == 0), stop=(kd == KD - 1))
                    ha = hadd.tile([P, MAXC], F16, tag="hadd", name=f"ha{n}")
                    nc.vector.tensor_tensor(
                        ha[:], ph[:], addv1T[:, n * MAXC:(n + 1) * MAXC],
                        ALU.add)
                    h1n[n] = h1s.tile([P, MAXC], F16, tag="h1n",
                                      name=f"h1n{n}")
                    nc.scalar.activation(
                        h1n[n][:], ha[:],
                        AF.Identity if sim_safe else AF.Gelu)

                def emit_h2(n):
                    nch, nl = n // MCH, n % MCH
                    wchunk = w12_t[nch]
                    src = h1n.pop(n)
                    for c in range(CT):
                        for dd3 in range(DD3):
                            nc.tensor.matmul(
                                h2ps[c][dd3][:],
                                src[:, c * P:(c + 1) * P],
                                wchunk[:, nl * W12 + KD * P + dd3 * 512:
                                       nl * W12 + KD * P + (dd3 + 1) * 512],
                                start=(n == 0), stop=False)

                for n in range(N2 + 1):
                    if n < N2:
                        if n % MCH == 0 and n // MCH + 3 < N2 // MCH:
                            fetch_w12(n // MCH + 3)
                        emit_h1(n)
                    if n >= 1:
                        emit_h2(n - 1)

                # ---- b2 + LayerNorm
                def ln_c(c):
                    h2 = lnp.tile([P, D], F32, tag="h2", name=f"h2_{c}")
                    parts = []
                    for dd3 in range(DD3):
                        nc.tensor.matmul(
                            h2ps[c][dd3][:], ones_row[:],
                            b2row[:, dd3 * 512:(dd3 + 1) * 512],
                            start=False, stop=True)
                        pacc = lns.tile([P, 1], F32, tag="pacc",
                                        name=f"pacc{c}_{dd3}")
                        nc.scalar.activation(
                            h2[:, dd3 * 512:(dd3 + 1) * 512], h2ps[c][dd3][:],
                            AF.Identity, accum_out=pacc[:])
                        parts.append(pacc)
                    s01 = lns.tile([P, 1], F32, tag="s01", name=f"s01_{c}")
                    nc.vector.tensor_tensor(s01[:], parts[0][:], parts[1][:],
                                            ALU.add)
                    s012 = lns.tile([P, 1], F32, tag="s012", name=f"s012_{c}")
                    nc.vector.tensor_tensor(s012[:], s01[:], parts[2][:],
                                            ALU.add)
                    negmu = lns.tile([P, 1], F32, tag="negmu")
                    nc.vector.tensor_scalar_mul(negmu[:], s012[:], -1.0 / D)
                    ssq = lns.tile([P, 1], F32, tag="ssq")
                    sq = lnq.tile([P, D], F32, tag="sq", name=f"sq_{c}")
                    nc.scalar.activation(sq[:], h2[:], AF.Square,
                                         bias=negmu[:], accum_out=ssq[:])
                    std = lns.tile([P, 1], F32, tag="std")
                    nc.scalar.activation(std[:], ssq[:], AF.Sqrt,
                                         bias=eps_ln[:], scale=1.0 / D)
                    rstd = lns.tile([P, 1], F32, tag="rstd")
                    nc.vector.reciprocal(rstd[:], std[:])
                    t1 = lnp.tile([P, D], F32, tag="t1", name=f"t1_{c}")
                    for dd3 in range(DD3):
                        sl = slice(dd3 * 512, (dd3 + 1) * 512)
                        eng = nc.vector if dd3 != 1 else nc.gpsimd
                        nc.vector.tensor_scalar(t1[:, sl], h2[:, sl],
                                                negmu[:], rstd[:],
                                                ALU.add, ALU.mult)
                        eng.tensor_tensor(t1[:, sl], t1[:, sl], lngb[:, sl],
                                          ALU.mult)
                        eng.tensor_tensor(t1[:, sl], t1[:, sl], lnbb[:, sl],
                                          ALU.add)
                        nc.scalar.dma_start(
                            d["out"].ap()[c * P:(c + 1) * P, sl], t1[:, sl])

                ln_c(0)
                ln_c(1)


# ------------------------------------------------------------------ driver

def prepare_inputs(x, boundaries, in_proj_w, in_proj_b, out_w, out_b,
                   w1, b1, w2, b2, ln_g, ln_b, pos_enc, size_emb):
    """Host prep: returns (wt, in_maps) for the 8 cores."""
    x = np.asarray(x, dtype=np.float32)
    boundaries = np.asarray(boundaries, dtype=np.float32)
    segs = [_host_segments(boundaries[b]) for b in range(B)]
    wt = _window_tiles([s[0] for s in segs])

    shared, shared_f = _pack_weights(in_proj_w, in_proj_b, out_w, out_b,
                                     w1, b1, w2, b2, ln_g, ln_b,
                                     pos_enc, size_emb)
    in_maps = []
    for b in range(B):
        seg, valid, seg_c, lengths = segs[b]
        maskM, orm, addv1T = _host_per_batch(seg, valid, seg_c, lengths,
                                             wt, shared_f)
        m = dict(shared)
        # xT16 [P, KD*S]: row p, col kd*S + tok
        xTb = np.ascontiguousarray(
            x[b].T.reshape(KD, P, S).transpose(1, 0, 2).reshape(P, KD * S))
        m["xT16"] = xTb.astype(np.float16)
        x8, sx = _q8(xTb)
        m["xT8"] = x8
        m["qksc"] = np.broadcast_to(np.asarray(
            [INV_SD / (sx * shared["sq"]), 1.0 / (sx * shared["sk"])],
            np.float32), (P, 2)).copy()
        del m["sx_ref"], m["sq"], m["sk"]
        m["maskM"] = maskM
        m["orm"] = orm
        m["addv1T"] = addv1T
        in_maps.append(m)
    return wt, in_maps


_NC_CACHE = {}


def get_nc(wt):
    if wt not in _NC_CACHE:
        _NC_CACHE[wt] = build_nc(wt)
    return _NC_CACHE[wt]


def kernel(**inputs):
    wt, in_maps = prepare_inputs(**inputs)
    nc = get_nc(wt)
    res = run_bass_kernel_spmd(nc, in_maps, list(range(B)))
    out = np.stack([res.results[b]["out"] for b in range(B)], axis=0)
    return out.astype(np.float32)


# revision 33
# speedup vs baseline: 2.2610x; 1.6829x over previous
"""Trainium2 Bass kernel for nn_EnhancedChunkLayer (ragged_sequence).

Strategy: data-parallel over batch B=8 across 8 NeuronCores (one batch
element per core, weights replicated). Inside each core (all PE matmul
operands fp16; fp32 PSUM accumulation, softmax and LayerNorm math):

  - banded block-diagonal attention with a centered wt*128-wide column
    window; scores are computed TRANSPOSED (scoresT[wincol, row] =
    K_h-layout @ Q_h) so the exp'd probs feed attn@V directly as the
    stationary operand -- no PE transposes at all.
  - softmax: exp directly off the scores PSUM (ACT), then a MULTIPLICATIVE
    {0,1} fp16 mask on DVE (exp(s+m) == exp(s)*exp(m)); V carries an
    interleaved 129th ones-column per head so the denominator accumulates
    inside the attn@V matmul itself; per-(t) batched max(den,eps) +
    reciprocal on DVE via a strided AP over the den columns; the per-row
    1/den folds into the per-head ctx PSUM->SBUF copy (Pool engine,
    per-partition scalar).
  - QK-projection of head-group g+1 is emission-interleaved with the
    attention of group g, so the in-order PE always has projection matmuls
    to chew on while ACT/DVE run the softmax chain.
  - mean-pool FIRST (on ctx, via a host-built recip-scaled one-hot matmul),
    THEN the out-projection on [MAXC, D] instead of [S, D] (4x fewer rows).
    bv folds into ob' = bv @ Wo^T + ob on the host; ob', the size-embedding
    lookup (host gather) and pos_enc fold into one addvecT tensor.
  - chunk MLP streams W1/W2 in combined per-n chunks with h1(n) / h2(n-1)
    software-pipelined; exact-erf GELU on ACT; LayerNorm on chip.

Host does index bookkeeping only: cumsum of boundary indicators, mask /
scaled-one-hot construction, the size-embedding row gather, bias folds, and
weight layout packs (fp16).
"""

import math
from contextlib import ExitStack

import numpy as np

import concourse.bacc as bacc
import concourse.bass as bass
import concourse.mybir as mybir
from concourse import tile
from concourse.bass_utils import run_bass_kernel_spmd

F32 = mybir.dt.float32
F16 = mybir.dt.float16
F8 = mybir.dt.float8e4
DR = mybir.MatmulPerfMode.DoubleRow
AF = mybir.ActivationFunctionType
ALU = mybir.AluOpType
AX = mybir.AxisListType

B, S, D = 8, 1024, 1536
H, DH = 12, 128
MAXC, MAXSEQ = 256, 1024
THRESH = 0.85
P = 128
KD = D // P          # 12 contraction tiles over D
NT = S // P          # 8 row tiles over S
N2 = (2 * D) // P    # 24 tiles over hidden 2D
CT = MAXC // P       # 2 chunk tiles
DD3 = D // 512       # 3 free-dim 512 tiles over D
HG = 3               # heads per group
NG = H // HG         # 4 head groups
VW = 129             # per-head V block: 128 feats + 1 ones col (den)
HB = H * VW          # per-tile V block width
W12 = KD * P + D     # combined per-n W1 col block + W2 row block
MCH = 2              # n-tiles per streamed MLP chunk
DK2 = KD // 2        # 6 double-contraction tiles (K=256, fp8 DoubleRow)
INV_SD = 1.0 / math.sqrt(DH)
EPS_DEN = 2.0 ** -14  # fp16 min normal; keeps fully-masked rows finite


def _q8(a, target=160.0):
    """Quantize to e4m3 with a power-of-2 scale; returns (quant, scale)."""
    a = np.asarray(a, np.float32)
    m = float(np.max(np.abs(a))) or 1.0
    sc = 2.0 ** np.floor(np.log2(target / m))
    return np.ascontiguousarray((a * sc).astype(mybir.dt.np(F8))), sc


def _strided_ap(base, extra_off, dims):
    """AP over `base`'s tensor at base.offset+extra_off with free dims
    `dims` (list of [stride, count]); partition dim copied from base."""
    return bass.AP(tensor=base.tensor, offset=base.offset + extra_off,
                   ap=[list(base.ap[0])] + [list(d) for d in dims])


# ---------------------------------------------------------------- host prep

def _host_segments(boundaries_b):
    is_b = boundaries_b > THRESH
    seg = np.cumsum(is_b.astype(np.int64)) - 1
    valid = seg >= 0
    seg_c = np.where(valid & (seg < MAXC), seg, MAXC)
    lengths = np.bincount(seg_c, minlength=MAXC + 1)[:MAXC]
    return seg, valid, seg_c, lengths


def _window_tiles(seg_list):
    """Smallest odd tile-count window covering every chunk from any row tile."""
    wt = 3
    while True:
        if wt > NT:
            return NT
        pad = (wt - 1) // 2
        ok = True
        for seg in seg_list:
            for t in range(NT):
                ci = min(max(t - pad, 0), NT - wt)
                lo, hi = ci * P, ci * P + wt * P
                rows = np.arange(t * P, (t + 1) * P)
                segs = seg[rows]
                vmask = segs >= 0
                if not vmask.any():
                    continue
                cols = np.isin(seg, segs[vmask]) & (seg >= 0)
                idx = np.nonzero(cols)[0]
                if len(idx) and (idx[0] < lo or idx[-1] >= hi):
                    ok = False
                    break
            if not ok:
                break
        if ok:
            return wt
        wt += 2


def _host_per_batch(seg, valid, seg_c, lengths, wt, shared_f):
    """Per-batch tensors: multiplicative mask (transposed layout), scaled
    one-hot pool matrix, addvecT (pos_enc + size_emb gather + folded ob')."""
    pad = (wt - 1) // 2
    # maskM: [P, NT * wt * P]; cols t*(wt*P) + w*P + r; 1.0 where attends
    maskM = np.zeros((P, NT * wt * P), dtype=np.float32)
    for t in range(NT):
        ci = min(max(t - pad, 0), NT - wt)
        seg_r = seg[t * P:(t + 1) * P]            # [128] row segs
        row_ok = seg_r >= 0
        for w in range(wt):
            seg_w = seg[(ci + w) * P:(ci + w + 1) * P]   # [128] win segs
            m = (seg_w[:, None] == seg_r[None, :]) & row_ok[None, :]
            blk = maskM[:, t * wt * P + w * P: t * wt * P + (w + 1) * P]
            blk[m] = 1.0
    # O'r: [P, NT*MAXC]; orm[p, t*MAXC + c] = 1/len_c if seg_c[t*128+p]==c
    orm = np.zeros((P, NT * MAXC), dtype=np.float32)
    recip = (1.0 / np.maximum(lengths, 1)).astype(np.float32)
    for t in range(NT):
        sc = seg_c[t * P:(t + 1) * P]
        okm = sc < MAXC
        orm[np.nonzero(okm)[0], t * MAXC + sc[okm]] = recip[sc[okm]]
    # addvec1 = addvec @ W1.T + b1, via pre-multiplied tables (host gather
    # only): pe1 + (len>0) * (se1[size_idx] + ob1)  [MAXC, 2D]
    ne = (lengths > 0).astype(np.float32)[:, None]
    size_idx = np.minimum(lengths, MAXSEQ - 1)
    addvec1 = (shared_f["pe1"]
               + ne * (shared_f["se1"][size_idx] + shared_f["ob1"][None, :]))
    # addv1T [P, N2*MAXC]: col n*MAXC + c, row = hid within n
    addv1T = np.ascontiguousarray(
        addvec1.T.reshape(N2, P, MAXC).transpose(1, 0, 2)
        .reshape(P, N2 * MAXC).astype(np.float16))
    return maskM.astype(np.float16), orm.astype(np.float16), addv1T


def _pack_weights(in_proj_w, in_proj_b, out_w, out_b, w1, b1, w2, b2,
                  ln_g, ln_b, pos_enc, size_emb):
    f32 = np.float32
    ipw = np.asarray(in_proj_w, f32)
    wqT = ipw[0:D].T            # [din, dout]
    wkT = ipw[D:2 * D].T
    wvT = ipw[2 * D:3 * D].T
    woT = np.asarray(out_w, f32).T
    w1T = np.asarray(w1, f32).T       # [D, 2D]
    w2T = np.asarray(w2, f32).T       # [2D, D]

    def h16(a):
        return np.ascontiguousarray(a.astype(np.float16))

    # wv16 [DD3*P, KD*512]: row dd3*P+p, col kd*512+c
    wv16 = h16(wvT.reshape(KD, P, DD3, 512).transpose(2, 1, 0, 3)
               .reshape(DD3 * P, KD * 512))
    # wqk8 [NG*P, HG*2*KD*P] fp8: row g*P+p,
    # col hl*(2*KD*P) + qk*(KD*P) + dk*(2*P) + half*P + m  (DoubleRow A/B)
    wq8, sq = _q8(wqT)
    wk8, sk = _q8(wkT)
    wq6 = np.asarray(wq8).reshape(DK2, 2, P, NG, HG, P)  # dk half p g hl m
    wk6 = np.asarray(wk8).reshape(DK2, 2, P, NG, HG, P)
    wqk = np.stack([wq6, wk6], axis=0)           # qk dk half p g hl m
    wqk8 = np.ascontiguousarray(
        wqk.transpose(4, 3, 5, 0, 1, 2, 6)       # g p hl qk dk half m
        .reshape(NG * P, HG * 2 * KD * P))
    # fold the out-projection into W1: W1' = w1 @ out_w  [2D, D]
    w1f = np.asarray(w1, f32) @ np.asarray(out_w, f32)
    w1fT = w1f.T                      # [D, 2D]
    # w1216 [P, N2*W12]: per n, [KD*P w1'-lhsT cols | D w2-rhs cols]
    w1p = w1fT.reshape(KD, P, N2, P).transpose(2, 1, 0, 3).reshape(
        N2, P, KD * P)
    w2p = w2T.reshape(N2, P, D)
    w1216 = h16(np.concatenate([w1p, w2p], axis=2)    # n p (KD*P + D)
                .transpose(1, 0, 2).reshape(P, N2 * W12))

    bq = np.asarray(in_proj_b[0:D], f32)
    bk = np.asarray(in_proj_b[D:2 * D], f32)
    bv = np.asarray(in_proj_b[2 * D:3 * D], f32)
    ob = np.asarray(out_b, f32)
    obp = np.asarray(out_w, f32) @ bv + ob       # folded ob' = bv @ Wo^T + ob

    shared = {
        "wv16": wv16, "wqk8": wqk8, "w1216": w1216,
        "sx_ref": np.zeros(1, np.float32),  # filled by prepare_inputs
        "sq": sq, "sk": sk,
        "bqs": np.ascontiguousarray(bq.reshape(H, P).T * INV_SD),
        "bkp": np.ascontiguousarray(bk.reshape(H, P).T),
        "b2row": h16(np.asarray(b2, f32).reshape(1, D)),
        "lngb": np.ascontiguousarray(
            np.broadcast_to(np.asarray(ln_g, f32), (P, D))),
        "lnbb": np.ascontiguousarray(
            np.broadcast_to(np.asarray(ln_b, f32), (P, D))),
    }
    w1Tf = np.asarray(w1, f32).T
    b1f = np.asarray(b1, f32)
    shared_f = {
        "pe1": np.asarray(pos_enc, f32).reshape(MAXC, D) @ w1Tf + b1f,
        "se1": np.asarray(size_emb, f32) @ w1Tf,
        "ob1": obp @ w1Tf,
    }
    return shared, shared_f


# ------------------------------------------------------------- device build

def build_nc(wt, sim_safe=False, repeat=1):
    WB = wt * P
    nc = bacc.Bacc("TRN2", target_bir_lowering=False, debug=False)
    dp = nc.declare_dram_parameter
    dram = {
        "xT16": dp("xT16", [P, KD * S], F16, isOutput=False),
        "xT8": dp("xT8", [P, KD * S], F8, isOutput=False),
        "wv16": dp("wv16", [DD3 * P, KD * 512], F16, isOutput=False),
        "wqk8": dp("wqk8", [NG * P, HG * 2 * KD * P], F8, isOutput=False),
        "qksc": dp("qksc", [P, 2], F32, isOutput=False),
        "w1216": dp("w1216", [P, N2 * W12], F16, isOutput=False),
        "maskM": dp("maskM", [P, NT * WB], F16, isOutput=False),
        "orm": dp("orm", [P, NT * MAXC], F16, isOutput=False),
        "addv1T": dp("addv1T", [P, N2 * MAXC], F16, isOutput=False),
        "bqs": dp("bqs", [P, H], F32, isOutput=False),
        "bkp": dp("bkp", [P, H], F32, isOutput=False),
        "b2row": dp("b2row", [1, D], F16, isOutput=False),
        "lngb": dp("lngb", [P, D], F32, isOutput=False),
        "lnbb": dp("lnbb", [P, D], F32, isOutput=False),
        "out": dp("out", [MAXC, D], F32, isOutput=True),
    }
    with ExitStack() as octx:
        tc = octx.enter_context(tile.TileContext(nc))
        for _rep in range(repeat):
            _emit(nc, tc, wt, sim_safe, dram)
    nc.finalize()
    return nc


def _emit_qk_block(nc, xT8, wqk, qt, kt, bqs, bkp, qksc, psqk, g, j):
    """One QK psum block: j indexes (hl, qk, half); 6 fp8 DoubleRow
    matmuls (K=256 each) + ACT copy (descale + bias)."""
    hl, qk, half = j // 4, (j // 2) % 2, j % 2
    h = g * HG + hl
    dst, bias = (qt, bqs) if qk == 0 else (kt, bkp)
    pq = psqk.tile([P, 512], F32, tag="pq", name=f"pq{g}_{j}")
    for dk in range(DK2):
        wb = wqk[:, hl * 2 * KD * P + qk * KD * P + dk * 2 * P:
                 hl * 2 * KD * P + qk * KD * P + dk * 2 * P + 1]
        xb = xT8[:, dk * 2 * S + half * 512: dk * 2 * S + half * 512 + 1]
        nc.tensor.matmul(
            pq[:],
            _strided_ap(wb, 0, [[P, 2], [1, P]]),
            _strided_ap(xb, 0, [[S, 2], [1, 512]]),
            start=(dk == 0), stop=(dk == DK2 - 1), perf_mode=DR)
    out_sl = dst[:, hl * S + half * 512: hl * S + (half + 1) * 512]
    if qk == 0:
        nc.scalar.activation(out_sl, pq[:], AF.Identity,
                             bias=bias[:, h:h + 1],
                             scale=qksc[:, qk:qk + 1])
    else:
        nc.vector.tensor_scalar(out_sl, pq[:], qksc[:, qk:qk + 1],
                                bias[:, h:h + 1], ALU.mult, ALU.add)


def _emit(nc, tc, wt, sim_safe, dram):
    d = dram
    WB = wt * P
    pad = (wt - 1) // 2
    ct_idx = [min(max(t - pad, 0), NT - wt) for t in range(NT)]
    DEPTH = 4  # attention software-pipeline depth (exp/mask latency hiding)

    with ExitStack() as ctx:
        persist = ctx.enter_context(tc.tile_pool(name="persist", bufs=1))

        xp_ctx = ExitStack()
        # pool-open order = SBUF stack order; wvs first so its early-freed
        # range (V-proj done ~25% in) sits where stage-3 tensors land.
        wvs = xp_ctx.enter_context(tc.tile_pool(name="wvs", bufs=2))
        xp = xp_ctx.enter_context(tc.tile_pool(name="xp", bufs=1))
        # DMA priority order: xT (everything waits on it), QK weights +
        # biases for group 0, V weights, then the rest.
        xT8 = xp.tile([P, KD * S], F8, tag="xT8")
        nc.sync.dma_start(xT8[:], d["xT8"].ap()[:])
        bqs = persist.tile([P, H], F32, tag="bqs")
        bkp = persist.tile([P, H], F32, tag="bkp")
        qksc = persist.tile([P, 2], F32, tag="qksc")
        xT = xp.tile([P, KD * S], F16, tag="xT")

        # ============ stage 1: V-proj; pipelined QK-proj + attention =======
        with tc.tile_pool(name="wqks", bufs=2) as wqks, \
             tc.tile_pool(name="qkp", bufs=2) as qkp, \
             tc.tile_pool(name="psqk", bufs=2, space="PSUM") as psqk, \
             tc.tile_pool(name="aw", bufs=DEPTH + 2) as aw, \
             tc.tile_pool(name="smallp", bufs=4) as smallp:

            # weight DMAs: first two groups' QK weights + V weights up
            # front (no WAR waits; SP queue flows); later groups fetched
            # inside the loop once their double-buffer slot frees.
            wqk_t = {}
            HLW = KD * 2 * P

            def fetch_wqk(g, split=False):
                w_ = wqks.tile([P, HG * HLW], F8, tag="wqk",
                               name=f"wqk{g}")
                if split:
                    for hl in range(HG):
                        nc.sync.dma_start(
                            w_[:, hl * HLW:(hl + 1) * HLW],
                            d["wqk8"].ap()[g * P:(g + 1) * P,
                                           hl * HLW:(hl + 1) * HLW])
                else:
                    nc.sync.dma_start(w_[:],
                                      d["wqk8"].ap()[g * P:(g + 1) * P, :])
                wqk_t[g] = w_

            fetch_wqk(0, split=True)
            nc.sync.dma_start(bqs[:], d["bqs"].ap()[:])
            nc.sync.dma_start(bkp[:], d["bkp"].ap()[:])
            nc.sync.dma_start(qksc[:], d["qksc"].ap()[:])
            nc.sync.dma_start(xT[:], d["xT16"].ap()[:])
            wv_t = []
            for dd3 in range(DD3):
                w_ = wvs.tile([P, KD * 512], F16, tag="wv", name=f"wv{dd3}")
                nc.sync.dma_start(w_[:], d["wv16"].ap()[dd3 * P:(dd3 + 1) * P, :])
                wv_t.append(w_)
            fetch_wqk(1)

            # big tensors threaded across stages + small consts
            v16 = persist.tile([P, NT * HB], F16, tag="v16")
            vap = v16[:, 0:1]
            nc.vector.memset(_strided_ap(vap, P, [[VW, NT * H], [1, 1]]), 1.0)
            ctx16 = persist.tile([P, NT * D], F16, tag="ctx16")
            maskM = persist.tile([P, NT * WB], F16, tag="maskM")
            nc.sync.dma_start(maskM[:], d["maskM"].ap()[:])
            ones_row = persist.tile([1, P], F16, tag="ones_row")
            nc.vector.memset(ones_row[:], 1.0)
            b2row = persist.tile([1, D], F16, tag="b2row")
            nc.sync.dma_start(b2row[:], d["b2row"].ap()[:])
            eps_ln = persist.tile([P, 1], F32, tag="eps_ln")
            nc.vector.memset(eps_ln[:], 1e-5)

            qts, kts = {}, {}

            def new_qk(g):
                qts[g] = qkp.tile([P, HG * S], F16, tag="qt", name=f"qt{g}")
                kts[g] = qkp.tile([P, HG * S], F16, tag="kt", name=f"kt{g}")

            # QK-proj of group 0 first (pure PE streak right after xT lands)
            new_qk(0)
            for j in range(4 * HG):
                _emit_qk_block(nc, xT8, wqk_t[0], qts[0], kts[0], bqs, bkp,
                               qksc, psqk, 0, j)

            # V-proj (must complete before attnV of group 0)
            with tc.tile_pool(name="psv", bufs=3, space="PSUM") as psv:
                for dd3 in range(DD3):
                    for mt in range(NT):
                        pv = psv.tile([P, 512], F32, tag="pv")
                        for kd in range(KD):
                            nc.tensor.matmul(
                                pv[:],
                                xT[:, kd * S + mt * P: kd * S + (mt + 1) * P],
                                wv_t[dd3][:, kd * 512:(kd + 1) * 512],
                                start=(kd == 0), stop=(kd == KD - 1))
                        vbase = v16[:, mt * HB + dd3 * 4 * VW:
                                    mt * HB + dd3 * 4 * VW + 1]
                        nc.scalar.activation(
                            _strided_ap(vbase, 0, [[VW, 4], [1, P]]),
                            pv[:].rearrange("p (h m) -> p h m", m=P),
                            AF.Identity)
            atn_ctx = ExitStack()
            pssc = atn_ctx.enter_context(
                tc.tile_pool(name="pssc", bufs=3, space="PSUM"))
            pscx = atn_ctx.enter_context(
                tc.tile_pool(name="pscx", bufs=2, space="PSUM"))

            # attention(g) emission-interleaved with QK-proj(g+1)
            def attn_emit(state):
                """Advance the attention software pipeline by one slot."""
                g = state["g"]
                i = state["i"]
                items = state["items"]
                if i < len(items):
                    t, hl = items[i]
                    ci = ct_idx[t]
                    if hl == 0:
                        state["pc"][t] = pscx.tile(
                            [P, HG * VW], F32, tag="pc", name=f"pc{g}_{t}")
                    sc = pssc.tile([P, WB], F32, tag="sc", name=f"sc{g}_{i}")
                    qt, kt = qts[g], kts[g]
                    for w in range(wt):
                        nc.tensor.matmul(
                            sc[:, w * P:(w + 1) * P],
                            kt[:, hl * S + (ci + w) * P:
                               hl * S + (ci + w + 1) * P],
                            qt[:, hl * S + t * P: hl * S + (t + 1) * P],
                            start=True, stop=True)
                    ex = aw.tile([P, WB], F16, tag="ex", name=f"ex{g}_{i}")
                    nc.scalar.activation(ex[:], sc[:], AF.Exp)
                    nc.vector.tensor_tensor(
                        ex[:], ex[:], maskM[:, t * WB:(t + 1) * WB], ALU.mult)
                    state["ex"][i] = ex
                if i >= DEPTH and i - DEPTH < len(items):
                    t, hl = items[i - DEPTH]
                    h = g * HG + hl
                    ci = ct_idx[t]
                    ex = state["ex"].pop(i - DEPTH)
                    pc = state["pc"][t]
                    for w in range(wt):
                        nc.tensor.matmul(
                            pc[:, hl * VW:(hl + 1) * VW],
                            ex[:, w * P:(w + 1) * P],
                            v16[:, (ci + w) * HB + h * VW:
                                (ci + w) * HB + (h + 1) * VW],
                            start=(w == 0), stop=(w == wt - 1))
                    if hl == HG - 1:
                        # batched max(den,eps) + recip for the 3 heads
                        dmx = smallp.tile([P, HG], F32, tag="dmx",
                                          name=f"dmx{g}_{t}")
                        nc.vector.tensor_scalar_max(
                            dmx[:], _strided_ap(pc[:, 0:1], P, [[VW, HG]]),
                            EPS_DEN)
                        rden = smallp.tile([P, HG], F32, tag="rden",
                                           name=f"rden{g}_{t}")
                        nc.vector.reciprocal(rden[:], dmx[:])
                        for hl2 in range(HG):
                            h2_ = g * HG + hl2
                            nc.vector.tensor_scalar_mul(
                                ctx16[:, t * D + h2_ * P:
                                      t * D + (h2_ + 1) * P],
                                pc[:, hl2 * VW:hl2 * VW + P],
                                rden[:, hl2:hl2 + 1])
                state["i"] += 1

            for g in range(NG):
                if g + 2 < NG:
                    fetch_wqk(g + 2)
                state = {"g": g, "i": 0, "ex": {}, "pc": {},
                         "items": [(t, hl) for t in range(NT)
                                   for hl in range(HG)]}
                n_steps = len(state["items"]) + DEPTH
                if g + 1 < NG:
                    new_qk(g + 1)
                    # interleave: 12 QK blocks across the 24+DEPTH attn slots
                    qk_jobs = list(range(4 * HG))
                    for step in range(n_steps):
                        if step % 2 == 0 and qk_jobs:
                            j = qk_jobs.pop(0)
                            _emit_qk_block(nc, xT8, wqk_t[g + 1], qts[g + 1],
                                           kts[g + 1], bqs, bkp, qksc, psqk,
                                           g + 1, j)
                        attn_emit(state)
                    for j in qk_jobs:
                        _emit_qk_block(nc, xT8, wqk_t[g + 1], qts[g + 1],
                                       kts[g + 1], bqs, bkp, qksc, psqk,
                                       g + 1, j)
                else:
                    for step in range(n_steps):
                        attn_emit(state)
            atn_ctx.close()
        xp_ctx.close()

        # ============ stage 2+3: pooling -> pooledT; fused MLP =============
        # (out-projection is folded into W1 on the host: W1' = W1 @ Wo,
        # addvec1 = addvec @ W1.T + b1)
        with tc.tile_pool(name="w12s", bufs=3) as w12s, \
             tc.tile_pool(name="adp", bufs=1) as adp:
            addv1T = adp.tile([P, N2 * MAXC], F16, tag="addv1T")
            nc.sync.dma_start(addv1T[:], d["addv1T"].ap()[:])
            lngb = persist.tile([P, D], F32, tag="lngb")
            nc.sync.dma_start(lngb[:], d["lngb"].ap()[:])
            lnbb = persist.tile([P, D], F32, tag="lnbb")
            nc.sync.dma_start(lnbb[:], d["lnbb"].ap()[:])
            pooledT = persist.tile([P, KD * MAXC], F16, tag="pooledT")
            orm = persist.tile([P, NT * MAXC], F16, tag="orm")
            nc.sync.dma_start(orm[:], d["orm"].ap()[:])
            w12_t = {}

            def fetch_w12(nch):
                w12_t[nch] = w12s.tile([P, MCH * W12], F16, tag="w12",
                                       name=f"w12c{nch}")
                nc.sync.dma_start(
                    w12_t[nch][:],
                    d["w1216"].ap()[:, nch * MCH * W12:(nch + 1) * MCH * W12])

            fetch_w12(0)
            fetch_w12(1)
            fetch_w12(2)
            with tc.tile_pool(name="pspool", bufs=2, space="PSUM") as pspool:
                for fd in range(KD):
                    pp = pspool.tile([P, MAXC], F32, tag="pp")
                    for t in range(NT):
                        nc.tensor.matmul(
                            pp[:],
                            ctx16[:, t * D + fd * P: t * D + (fd + 1) * P],
                            orm[:, t * MAXC:(t + 1) * MAXC],
                            start=(t == 0), stop=(t == NT - 1))
                    nc.scalar.activation(
                        pooledT[:, fd * MAXC:(fd + 1) * MAXC], pp[:],
                        AF.Identity)

            # ======== stage 4+5: fused MLP h1(n)/h2(n-1) pipeline ==========
            with tc.tile_pool(name="h1s", bufs=3) as h1s, \
                 tc.tile_pool(name="hadd", bufs=3) as hadd, \
                 tc.tile_pool(name="lnp", bufs=2) as lnp, \
                 tc.tile_pool(name="lnq", bufs=1) as lnq, \
                 tc.tile_pool(name="lns", bufs=6) as lns, \
                 tc.tile_pool(name="psh1", bufs=2, space="PSUM") as psh1, \
                 tc.tile_pool(name="psh2", bufs=1, space="PSUM") as psh2:
                h2ps = [[psh2.tile([P, 512], F32, tag=f"ph2_{c}_{j}",
                                   name=f"ph2_{c}_{j}")
                         for j in range(DD3)] for c in range(CT)]
                h1n = {}

                def emit_h1(n):
                    nch, nl = n // MCH, n % MCH
                    ph = psh1.tile([P, MAXC], F32, tag="ph", name=f"ph{n}")
                    wchunk = w12_t[nch]
                    for kd in range(KD):
                        nc.tensor.matmul(
                            ph[:],
                            wchunk[:, nl * W12 + kd * P:
                                   nl * W12 + (kd + 1) * P],
                            pooledT[:, kd * MAXC:(kd + 1) * MAXC],
                            start=(kd == 0), stop=(kd == KD - 1))
                    ha = hadd.tile([P, MAXC], F16, tag="hadd", name=f"ha{n}")
                    nc.vector.tensor_tensor(
                        ha[:], ph[:], addv1T[:, n * MAXC:(n + 1) * MAXC],
                        ALU.add)
                    h1n[n] = h1s.tile([P, MAXC], F16, tag="h1n",
                                      name=f"h1n{n}")
                    nc.scalar.activation(
                        h1n[n][:], ha[:],
                        AF.Identity if sim_safe else AF.Gelu)

                def emit_h2(n):
                    nch, nl = n // MCH, n % MCH
                    wchunk = w12_t[nch]
                    src = h1n.pop(n)
                    for c in range(CT):
                        for dd3 in range(DD3):
                            nc.tensor.matmul(
                                h2ps[c][dd3][:],
                                src[:, c * P:(c + 1) * P],
                                wchunk[:, nl * W12 + KD * P + dd3 * 512:
                                       nl * W12 + KD * P + (dd3 + 1) * 512],
                                start=(n == 0), stop=False)

                for n in range(N2 + 1):
                    if n < N2:
                        if n % MCH == 0 and n // MCH + 3 < N2 // MCH:
                            fetch_w12(n // MCH + 3)
                        emit_h1(n)
                    if n >= 1:
                        emit_h2(n - 1)

                # ---- b2 + LayerNorm
                def ln_c(c):
                    h2 = lnp.tile([P, D], F32, tag="h2", name=f"h2_{c}")
                    parts = []
                    for dd3 in range(DD3):
                        nc.tensor.matmul(
                            h2ps[c][dd3][:], ones_row[:],
                            b2row[:, dd3 * 512:(dd3 + 1) * 512],
                            start=False, stop=True)
                        pacc = lns.tile([P, 1], F32, tag="pacc",
                                        name=f"pacc{c}_{dd3}")
                        nc.scalar.activation(
                            h2[:, dd3 * 512:(dd3 + 1) * 512], h2ps[c][dd3][:],
                            AF.Identity, accum_out=pacc[:])
                        parts.append(pacc)
                    s01 = lns.tile([P, 1], F32, tag="s01", name=f"s01_{c}")
                    nc.vector.tensor_tensor(s01[:], parts[0][:], parts[1][:],
                                            ALU.add)
                    s012 = lns.tile([P, 1], F32, tag="s012", name=f"s012_{c}")
                    nc.vector.tensor_tensor(s012[:], s01[:], parts[2][:],
                                            ALU.add)
                    negmu = lns.tile([P, 1], F32, tag="negmu")
                    nc.vector.tensor_scalar_mul(negmu[:], s012[:], -1.0 / D)
                    ssq = lns.tile([P, 1], F32, tag="ssq")
                    sq = lnq.tile([P, D], F32, tag="sq", name=f"sq_{c}")
                    nc.scalar.activation(sq[:], h2[:], AF.Square,
                                         bias=negmu[:], accum_out=ssq[:])
                    std = lns.tile([P, 1], F32, tag="std")
                    nc.scalar.activation(std[:], ssq[:], AF.Sqrt,
                                         bias=eps_ln[:], scale=1.0 / D)
                    rstd = lns.tile([P, 1], F32, tag="rstd")
                    nc.vector.reciprocal(rstd[:], std[:])
                    t1 = lnp.tile([P, D], F32, tag="t1", name=f"t1_{c}")
                    for dd3 in range(DD3):
                        sl = slice(dd3 * 512, (dd3 + 1) * 512)
                        eng = nc.vector if dd3 != 1 else nc.gpsimd
                        nc.vector.tensor_scalar(t1[:, sl], h2[:, sl],
                                                negmu[:], rstd[:],
                                                ALU.add, ALU.mult)
                        eng.tensor_tensor(t1[:, sl], t1[:, sl], lngb[:, sl],
                                          ALU.mult)
                        eng.tensor_tensor(t1[:, sl], t1[:, sl], lnbb[:, sl],
                                          ALU.add)
                        nc.scalar.dma_start(
                            d["out"].ap()[c * P:(c + 1) * P, sl], t1[:, sl])

                ln_c(0)
                ln_c(1)


# ------------------------------------------------------------------ driver

def prepare_inputs(x, boundaries, in_proj_w, in_proj_b, out_w, out_b,
                   w1, b1, w2, b2, ln_g, ln_b, pos_enc, size_emb):
    """Host prep: returns (wt, in_maps) for the 8 cores."""
    x = np.asarray(x, dtype=np.float32)
    boundaries = np.asarray(boundaries, dtype=np.float32)
    segs = [_host_segments(boundaries[b]) for b in range(B)]
    wt = _window_tiles([s[0] for s in segs])

    shared, shared_f = _pack_weights(in_proj_w, in_proj_b, out_w, out_b,
                                     w1, b1, w2, b2, ln_g, ln_b,
                                     pos_enc, size_emb)
    in_maps = []
    for b in range(B):
        seg, valid, seg_c, lengths = segs[b]
        maskM, orm, addv1T = _host_per_batch(seg, valid, seg_c, lengths,
                                             wt, shared_f)
        m = dict(shared)
        # xT16 [P, KD*S]: row p, col kd*S + tok
        xTb = np.ascontiguousarray(
            x[b].T.reshape(KD, P, S).transpose(1, 0, 2).reshape(P, KD * S))
        m["xT16"] = xTb.astype(np.float16)
        x8, sx = _q8(xTb)
        m["xT8"] = x8
        m["qksc"] = np.broadcast_to(np.asarray(
            [INV_SD / (sx * shared["sq"]), 1.0 / (sx * shared["sk"])],
            np.float32), (P, 2)).copy()
        del m["sx_ref"], m["sq"], m["sk"]
        m["maskM"] = maskM
        m["orm"] = orm
        m["addv1T"] = addv1T
        in_maps.append(m)
    return wt, in_maps


_NC_CACHE = {}


def get_nc(wt):
    if wt not in _NC_CACHE:
        _NC_CACHE[wt] = build_nc(wt)
    return _NC_CACHE[wt]


def kernel(**inputs):
    wt, in_maps = prepare_inputs(**inputs)
    nc = get_nc(wt)
    res = run_bass_kernel_spmd(nc, in_maps, list(range(B)))
    out = np.stack([res.results[b]["out"] for b in range(B)], axis=0)
    return out.astype(np.float32)
